# revision 1
# baseline (speedup 1.0000x reference)
"""Multi-headed causal attention on 8 trn2 NeuronCores (Bass/Tile).

Sharding: tensor-parallel over heads — 2 heads per core, all 4 batches.
Per core:
  - Q^T/K^T/V^T projections with the 2 heads stacked on the partition axis
    (full 128-wide fp32r matmuls, contraction over D streamed from a
    host-side transposed copy of `embedded`).
  - scores computed transposed ([s_k, s_q] layout) in bf16 with K padded
    to 128 by zero rows (per-head K^T tiles carry zeros in the other
    head's partition range, so the stacked Q^T is the moving operand for
    both heads and every matmul is the fast full-array 128x128x512 shape).
  - both heads' score tiles land in one 2-bank PSUM tile; a single exp on
    ScalarE (no max subtraction: logits are ~N(0,1)); causal masking via
    one affine_select on GpSimd; always-masked leading columns of
    diagonal tiles are skipped in both the matmul and the exp.
  - AV matmul uses V padded to 128 columns (64 V | ones | zeros): softmax
    denominators fall out as row 64 of the context accumulator; context
    leaves PSUM immediately and is normalized off the critical path
    (fast-reciprocal on a partition-0 tile + GpSimd partition broadcast).
  - two 2MB AllToAlls redistribute normalized context by owner-column
    halves; each core then runs the row-sharded fp32r output projection
    (+bias) for its 1024 rows of (B*S).
Batch-major emission lets batch b+1's projections/DMA overlap batch b's
attention; a 2-deep score->exp->AV software pipeline keeps the PE array
busy inside the attention loop.
"""
import sys

sys.path.insert(0, "/opt/trn_rl_repo")

import numpy as np

import concourse.bass as bass
import concourse.tile as tile
from concourse import bacc, mybir
from concourse.bass_utils import run_bass_kernel_spmd

B, S, D, H, HD = 4, 2048, 1024, 16, 64
NC_ = 8          # cores
PH = 2           # heads per core
SC = 512         # s_q chunk (psum bank width in fp32)
NK = S // 128    # 16 s_k chunks of 128
ND = D // 128    # 8 contraction chunks of 128
F32 = mybir.dt.float32
F32R = mybir.dt.float32r
BF16 = mybir.dt.bfloat16
EXP = mybir.ActivationFunctionType.Exp
GE = mybir.AluOpType.is_ge


def build():
    nc = bacc.Bacc("TRN2", target_bir_lowering=False, debug=False, num_devices=NC_)

    emb_t = nc.dram_tensor("embedded_t", [B, D, S], F32R, kind="ExternalInput").ap()
    w_qkv = nc.dram_tensor("w_qkv", [3, ND, 128, 128], F32R, kind="ExternalInput").ap()
    wo_t = nc.dram_tensor("wo_t", [ND, 128, D], BF16, kind="ExternalInput").ap()
    bo_row = nc.dram_tensor("bo_row", [1, D], F32, kind="ExternalInput").ap()
    out_shard = nc.dram_tensor("out_shard", [1024, D], F32, kind="ExternalOutput").ap()

    with tile.TileContext(nc) as tc:
        _build_body(nc, tc, emb_t, w_qkv, wo_t, bo_row, out_shard)

    nc.compile()
    return nc


def _build_body(nc, tc, emb_t, w_qkv, wo_t, bo_row, out_shard):
    from contextlib import ExitStack

    ctx = ExitStack()
    with ctx:
        const = ctx.enter_context(tc.tile_pool(name="const", bufs=1))
        # "mm" slots are sized [128, 1024] (2 PSUM banks): 3x2 + ctx 2x1 = 8
        ps_mm = ctx.enter_context(tc.tile_pool(name="ps_mm", bufs=3, space="PSUM"))
        ps_ctx = ctx.enter_context(tc.tile_pool(name="ps_ctx", bufs=2, space="PSUM"))
        dram = ctx.enter_context(tc.tile_pool(name="dram", bufs=1, space="DRAM"))

        attn_ctx = ExitStack()
        etp = attn_ctx.enter_context(tc.tile_pool(name="etp", bufs=10))
        qtp = attn_ctx.enter_context(tc.tile_pool(name="qtp", bufs=2))
        ktp = attn_ctx.enter_context(tc.tile_pool(name="ktp", bufs=2))
        vtp = attn_ctx.enter_context(tc.tile_pool(name="vtp", bufs=2))
        vsb = attn_ctx.enter_context(tc.tile_pool(name="vsb", bufs=2))
        exp_p = attn_ctx.enter_context(tc.tile_pool(name="exp_p", bufs=6))
        cu_p = attn_ctx.enter_context(tc.tile_pool(name="cu_p", bufs=3))
        cn_p = attn_ctx.enter_context(tc.tile_pool(name="cn_p", bufs=3))
        rc_p = attn_ctx.enter_context(tc.tile_pool(name="rc_p", bufs=2))
        rb_p = attn_ctx.enter_context(tc.tile_pool(name="rb_p", bufs=2))

        # ---- prefetch batch 0 activations before anything else ----
        et_pre = {}
        for j4 in range(4):
            for c in range(ND):
                t = etp.tile([128, SC], F32R, tag="et", name=f"et0_{j4}_{c}")
                eng = nc.sync if (c % 2 == 0) else nc.scalar
                eng.dma_start(
                    out=t[:], in_=emb_t[0, 128 * c:128 * (c + 1),
                                        SC * j4:SC * (j4 + 1)])
                et_pre[(j4, c)] = t

        # ---- constants (wo_t loads happen in the output phase) ----
        # all 24 qkv weight chunks in one tile / one DMA
        wq_all = const.tile([128, 24, 128], F32R, tag="wq_all")
        for p in range(3):
            nc.sync.dma_start(out=wq_all[:, 8 * p:8 * (p + 1), :],
                              in_=bass.AP(
                tensor=w_qkv.tensor, offset=131072 * p,
                ap=[[128, 128], [16384, 8], [1, 128]]))
        wq_sb = [[wq_all[:, 8 * p + c, :] for c in range(ND)] for p in range(3)]

        bo_sb = const.tile([1, D], F32, tag="bo1")
        nc.sync.dma_start(out=bo_sb[:], in_=bo_row[:])
        bo_b = const.tile([128, D], F32, tag="bob")
        nc.gpsimd.partition_broadcast(bo_b[:], bo_sb[:])

        ones_f32 = const.tile([128, 1], F32, tag="ones_f32")
        nc.vector.memset(ones_f32[:], 1.0)
        ones_r = const.tile([128, 1], BF16, tag="ones_r")
        nc.vector.tensor_copy(ones_r[:], ones_f32[:])

        ident = const.tile([128, 128], F32, tag="ident")
        nc.gpsimd.memset(ident[:], 1.0)
        nc.gpsimd.affine_select(out=ident[:], in_=ident[:], compare_op=GE,
                                fill=0.0, base=0, pattern=[[-1, 128]],
                                channel_multiplier=1)
        nc.gpsimd.affine_select(out=ident[:], in_=ident[:], compare_op=GE,
                                fill=0.0, base=0, pattern=[[1, 128]],
                                channel_multiplier=-1)

        a2a_in = [dram.tile([NC_, 128, 512], BF16, tag=f"a2a_in{q}",
                            name=f"a2a_in{q}") for q in range(2)]
        a2a_out = [dram.tile([NC_, 128, 512], BF16, tag=f"a2a_out{q}",
                             name=f"a2a_out{q}") for q in range(2)]

        def emit_a2a(q):
            nc.gpsimd.collective_compute(
                "AllToAll", mybir.AluOpType.bypass,
                replica_groups=[list(range(NC_))],
                ins=[a2a_in[q].opt()], outs=[a2a_out[q].opt()])

        # ---- per-batch: projections + attention ----
        for b in range(B):
            if b == 0:
                et = et_pre
            else:
                et = {}
                for c in range(ND):
                    t = etp.tile([128, S], F32R, tag="et", name=f"et{b}_{c}")
                    nc.sync.dma_start(
                        out=t[:], in_=emb_t[b, 128 * c:128 * (c + 1), :])
                    et[c] = t

            qt = qtp.tile([128, S], BF16, tag="qt")
            # per-head K^T padded to K=128 with zero rows for the other head
            kt0 = ktp.tile([128, S], BF16, tag="kt0")
            kt1 = ktp.tile([128, S], BF16, tag="kt1")
            nc.vector.memset(kt0[64:128, :], 0.0)
            nc.vector.memset(kt1[0:64, :], 0.0)
            vt = vtp.tile([128, S], F32, tag="vt")
            for j4 in range(4):          # s chunks of 512
                sl = slice(SC * j4, SC * (j4 + 1))
                for p in range(3):
                    ps = ps_mm.tile([128, SC], F32, tag="mm",
                                    name=f"pj{b}_{j4}_{p}")
                    for c in range(ND):
                        rhs = (et[(j4, c)][:] if b == 0
                               else et[c][:, sl])
                        nc.tensor.matmul(
                            ps[:], lhsT=wq_sb[p][c], rhs=rhs,
                            start=(c == 0), stop=(c == ND - 1))
                    if p == 0:
                        nc.vector.tensor_copy(qt[:, sl], ps[:])
                    elif p == 1:
                        nc.vector.tensor_copy(kt0[0:64, sl], ps[0:64, :])
                        nc.vector.tensor_copy(kt1[64:128, sl], ps[64:128, :])
                    else:
                        nc.scalar.copy(vt[:, sl], ps[:])
            kts = [kt0, kt1]

            # V natural layout padded to 128 cols: V | ones | zeros
            v01 = [vsb.tile([128, NK, 128], BF16, tag=f"v{h}", name=f"v{h}")
                   for h in range(PH)]
            for h in range(PH):
                nc.vector.memset(v01[h][:, :, 65:128], 0.0)
            for sk in range(NK):
                pt = ps_mm.tile([128, 128], F32, tag="mm", name=f"tr{b}_{sk}")
                nc.tensor.transpose(pt[:], vt[:, 128 * sk:128 * (sk + 1)], ident[:])
                for h in range(PH):
                    nc.vector.tensor_copy(v01[h][:, sk, 0:64],
                                          pt[:, 64 * h:64 * (h + 1)])
                    nc.vector.tensor_copy(v01[h][:, sk, 64:65], ones_r[:])

            # attention, one s_q chunk of 512 at a time
            # even j first: owner-block cols [0:512) complete after the
            # last batch's j=2, letting A2A #0/#1 overlap the odd-j work
            for j in (0, 2, 1, 3):
                mtop = 4 * j + 4
                ctx_ps = [ps_ctx.tile([128, SC], F32, tag="ctx",
                                      name=f"ctx{b}_{j}_{h}")
                          for h in range(PH)]
                PIPE = 2   # scores run this many m-iterations ahead of AV
                exq = []   # (m, ex) awaiting AV

                def emit_scores(m):
                    # cols [0, c0) of each half are fully causal-masked
                    c0 = max(0, 128 * m - SC * j)
                    psc = ps_mm.tile([128, 2 * SC], F32, tag="mm",
                                     name=f"sc{b}_{j}_{m}")
                    for h in range(PH):
                        nc.tensor.matmul(
                            psc[:, SC * h + c0:SC * (h + 1)],
                            lhsT=kts[h][:, 128 * m:128 * (m + 1)],
                            rhs=qt[:, SC * j + c0:SC * (j + 1)],
                            start=True, stop=True)
                    ex = exp_p.tile([128, 2 * SC], BF16, tag="ex",
                                    name=f"ex{b}_{j}_{m}")
                    nc.scalar.activation(out=ex[:, c0:], in_=psc[:, c0:],
                                         func=EXP, scale=0.125)
                    if m >= 4 * j:  # diagonal tile: zero k>q entries
                        nc.gpsimd.affine_select(
                            out=ex[:], in_=ex[:], compare_op=GE, fill=0.0,
                            base=SC * j - 128 * m, pattern=[[0, 2], [1, SC]],
                            channel_multiplier=-1)
                    exq.append((m, ex))

                def emit_av():
                    m_av, ex = exq.pop(0)
                    for h in range(PH):
                        nc.tensor.matmul(
                            ctx_ps[h][:], lhsT=v01[h][:, m_av, :],
                            rhs=ex[:, SC * h:SC * (h + 1)],
                            start=(m_av == 0), stop=(m_av == mtop - 1))

                for m in range(mtop):
                    emit_scores(m)
                    if len(exq) > PIPE:
                        emit_av()
                while exq:
                    emit_av()

                # free PSUM fast, then normalize off the critical path
                # owner-block col half: even j -> buffer 0, odd j -> 1
                o, q0 = 2 * b + j // 2, j % 2
                for h in range(PH):
                    cu = cu_p.tile([64, SC], F32, tag="cu",
                                   name=f"cu{b}_{j}_{h}")
                    nc.scalar.copy(cu[:], ctx_ps[h][0:64, :])
                    dn = rc_p.tile([1, SC], F32, tag="dn",
                                   name=f"dn{b}_{j}_{h}")
                    nc.scalar.copy(dn[:], ctx_ps[h][64:65, :])
                    rc = rc_p.tile([1, SC], F32, tag="rc")
                    nc.vector.reciprocal_approx_fast(rc[:], dn[:])
                    rb = rb_p.tile([64, SC], F32, tag="rb")
                    nc.gpsimd.partition_broadcast(rb[:], rc[:])
                    cn = cn_p.tile([64, SC], BF16, tag="cn")
                    nc.vector.tensor_mul(cn[:], cu[0:64, :], rb[:])
                    hr = slice(64 * h, 64 * (h + 1))
                    nc.sync.dma_start(out=a2a_in[q0][o, hr, :], in_=cn[:])
                if b == B - 1 and j == 2:
                    emit_a2a(0)      # even-j halves complete; overlaps j1/j3

        # ---- remaining all-to-all + row-sharded output projection ----
        attn_ctx.close()
        emit_a2a(1)

        cat_p = ctx.enter_context(tc.tile_pool(name="cat_p", bufs=16))
        ob_p = ctx.enter_context(tc.tile_pool(name="ob_p", bufs=3))
        wot_sb = [const.tile([128, D], BF16, tag=f"wo{c}", name=f"wo{c}")
                  for c in range(ND)]
        for c in range(ND):
            nc.sync.dma_start(out=wot_sb[c][:], in_=wo_t[c])
        for q in range(2):
            cats = []
            for r in range(NC_):
                ct = cat_p.tile([128, 512], BF16, tag=f"cat{q}",
                                name=f"cat{q}_{r}")
                nc.sync.dma_start(out=ct[:], in_=a2a_out[q][r])
                cats.append(ct)
            for sq in (4 * q, 4 * q + 1, 4 * q + 2, 4 * q + 3):
                lo = 128 * (sq % 4)
                for n in range(2):
                    po = ps_mm.tile([128, SC], F32, tag="mm",
                                    name=f"po{sq}_{n}")
                    for kp in range(ND):
                        nc.tensor.matmul(
                            po[:], lhsT=cats[kp][:, lo:lo + 128],
                            rhs=wot_sb[kp][:, SC * n:SC * (n + 1)],
                            start=(kp == 0), stop=(kp == ND - 1))
                    ob = ob_p.tile([128, SC], F32, tag="ob")
                    nc.vector.tensor_add(ob[:], po[:],
                                         bo_b[:, SC * n:SC * (n + 1)])
                    nc.sync.dma_start(
                        out=out_shard[128 * sq:128 * (sq + 1),
                                      SC * n:SC * (n + 1)],
                        in_=ob[:])


_NC_CACHE = None


def _get_nc():
    global _NC_CACHE
    if _NC_CACHE is None:
        _NC_CACHE = build()
    return _NC_CACHE


def _round_fp32r(x):
    """Round fp32 to fp32r (11-bit mantissa, RNE) — what the PE expects."""
    u = np.ascontiguousarray(x, np.float32).view(np.uint32)
    r = (u + np.uint32(0x7FF) + ((u >> np.uint32(12)) & np.uint32(1))) & np.uint32(0xFFFFF000)
    return r.view(np.float32)


def kernel(embedded, Wq, Wk, Wv, Wo, bo, _trace=False):
    embedded = np.ascontiguousarray(np.asarray(embedded, np.float32))
    emb_t = _round_fp32r(np.ascontiguousarray(embedded.transpose(0, 2, 1)))
    W = _round_fp32r(np.stack([np.asarray(Wq), np.asarray(Wk), np.asarray(Wv)]).astype(np.float32))
    import ml_dtypes
    wo_t = np.ascontiguousarray(np.asarray(Wo, np.float32).T).astype(
        ml_dtypes.bfloat16).reshape(ND, 128, D)
    bo_row = np.asarray(bo, np.float32).reshape(1, D)

    in_maps = []
    for c in range(NC_):
        w = W[:, 2 * c:2 * c + 2]                  # [3, 2, D, HD]
        w = np.ascontiguousarray(w.transpose(0, 2, 1, 3)).reshape(3, ND, 128, 128)
        in_maps.append({
            "embedded_t": emb_t,
            "w_qkv": w,
            "wo_t": wo_t,
            "bo_row": bo_row,
        })

    nc = _get_nc()
    res = run_bass_kernel_spmd(nc, in_maps, core_ids=list(range(NC_)),
                               trace=_trace)

    out = np.empty((B, S, D), np.float32)
    for c in range(NC_):
        s0 = (c % 2) * 1024
        out[c // 2, s0:s0 + 1024, :] = res.results[c]["out_shard"]
    if _trace:
        return out, res
    return out



# revision 4
# speedup vs baseline: 1.3488x; 1.3488x over previous
"""Multi-headed causal attention on 8 trn2 NeuronCores (Bass/Tile).

Sharding: batch per core-PAIR, heads split within the pair.
Core c: batch c//2, heads 8*(c%2) .. 8*(c%2)+8 (4 pblocks of 2 heads),
output columns 512*(c%2) .. +512 of its batch (col-split of Wo).

All per-core differences are INPUT DATA (emb of its batch, its 8 heads'
Wq/Wk/Wv, its 512 Wo columns + bias slice); the SPMD program is
identical, so cores never need role-dependent addressing.

Per core:
  - Q^T/K^T/V^T projections per pblock (2 heads stacked on 128
    partitions), contraction over D streamed from bf16 emb chunks.
  - scores transposed ([s_k, s_q]) in bf16, per-head K=64 matmuls at
    partition offset 64*h (PE tile_position) — no zero-padding.
  - exp on ScalarE (no max subtraction; logits ~N(0,1)); causal mask
    via one affine_select on GpSimd; always-masked leading columns of
    diagonal tiles skipped in matmul+exp.
  - AV with V padded to 65 cols (64 V | ones): softmax denominators
    fall out as row 64 of the context accumulator; normalize via
    fast-reciprocal + GpSimd partition broadcast.
  - Per-pblock 2-core AllGather (groups [[0,1],[2,3],[4,5],[6,7]])
    exchanges normalized context within the pair — only pair-local
    launch skew is absorbed, and each AG overlaps the next pblock's
    attention.
  - Output projection: my 512 Wo columns for ALL 2048 rows of the
    batch, contraction over the full concat dim read back from the
    AllGather outputs (slot order = natural head order, so no role
    dependence).
Proj and attention are interleaved per pblock so ScalarE exp work
overlaps the next pblock's projections; a 2-deep score->exp->AV
pipeline keeps the PE busy inside the attention loop.
"""
import sys

sys.path.insert(0, "/opt/trn_rl_repo")

import numpy as np

import concourse.bass as bass
import concourse.tile as tile
from concourse import bacc, mybir
from concourse.bass_utils import run_bass_kernel_spmd

B, S, D, H, HD = 4, 2048, 1024, 16, 64
NC_ = 8          # cores
NPB = 4          # pblocks per core (2 heads each -> 8 heads)
SC = 512         # s_q chunk (psum bank width in fp32)
NK = S // 128    # 16 s_k chunks of 128
ND = D // 128    # 8 contraction chunks of 128
F32 = mybir.dt.float32
BF16 = mybir.dt.bfloat16
EXP = mybir.ActivationFunctionType.Exp
GE = mybir.AluOpType.is_ge
PAIRS = [[0, 1], [2, 3], [4, 5], [6, 7]]


def build():
    nc = bacc.Bacc("TRN2", target_bir_lowering=False, debug=False, num_devices=NC_)

    emb_t = nc.dram_tensor("emb_t", [ND, 128, S], BF16, kind="ExternalInput").ap()
    w_qkv = nc.dram_tensor("w_qkv", [3, NPB, ND, 128, 128], BF16,
                           kind="ExternalInput").ap()
    wo_t = nc.dram_tensor("wo_t", [ND, 128, SC], BF16, kind="ExternalInput").ap()
    bo_col = nc.dram_tensor("bo_col", [1, SC], F32, kind="ExternalInput").ap()
    out_shard = nc.dram_tensor("out_shard", [S, SC], F32, kind="ExternalOutput").ap()

    with tile.TileContext(nc) as tc:
        _build_body(nc, tc, emb_t, w_qkv, wo_t, bo_col, out_shard)

    nc.compile()
    return nc


def _build_body(nc, tc, emb_t, w_qkv, wo_t, bo_col, out_shard):
    from contextlib import ExitStack

    ctx = ExitStack()
    with ctx:
        const = ctx.enter_context(tc.tile_pool(name="const", bufs=1))
        # "mm" slots are [128, 1024] fp32 (2 PSUM banks): 3x2 + ctx 2x1 = 8
        ps_mm = ctx.enter_context(tc.tile_pool(name="ps_mm", bufs=3, space="PSUM"))
        ps_ctx = ctx.enter_context(tc.tile_pool(name="ps_ctx", bufs=2, space="PSUM"))
        dram = ctx.enter_context(tc.tile_pool(name="dram", bufs=1, space="DRAM"))

        etp = ctx.enter_context(tc.tile_pool(name="etp", bufs=1))
        qtp = ctx.enter_context(tc.tile_pool(name="qtp", bufs=1))
        ktp = ctx.enter_context(tc.tile_pool(name="ktp", bufs=1))
        vtp = ctx.enter_context(tc.tile_pool(name="vtp", bufs=2))
        vsb = ctx.enter_context(tc.tile_pool(name="vsb", bufs=2))
        exp_p = ctx.enter_context(tc.tile_pool(name="exp_p", bufs=6))
        cu_p = ctx.enter_context(tc.tile_pool(name="cu_p", bufs=3))
        cn_p = ctx.enter_context(tc.tile_pool(name="cn_p", bufs=3))
        rc_p = ctx.enter_context(tc.tile_pool(name="rc_p", bufs=2))
        rb_p = ctx.enter_context(tc.tile_pool(name="rb_p", bufs=2))
        cat_p = ctx.enter_context(tc.tile_pool(name="cat_p", bufs=8))
        ob_p = ctx.enter_context(tc.tile_pool(name="ob_p", bufs=3))

        # ---- input DMAs ----
        # emb chunks split into 1024-col halves; j2=0 halves first so the
        # first projection slot can start before the full 4MB lands.
        et = {}
        for c in range(ND):
            t = etp.tile([128, S], BF16, tag=f"et{c}", name=f"et{c}")
            et[c] = t
        for j2 in range(2):
            sl = slice(1024 * j2, 1024 * (j2 + 1))
            for c in range(ND):
                eng = nc.sync if (c % 2 == 0) else nc.scalar
                eng.dma_start(out=et[c][:, sl], in_=emb_t[c, :, sl])

        w_sb = const.tile([128, 3 * NPB * ND, 128], BF16, tag="w_sb")
        for p in range(3):
            for pb in range(NPB):
                i = NPB * p + pb
                nc.scalar.dma_start(
                    out=w_sb[:, ND * i:ND * (i + 1), :],
                    in_=bass.AP(tensor=w_qkv.tensor,
                                offset=(i * ND) * 128 * 128,
                                ap=[[128, 128], [16384, ND], [1, 128]]))

        def wq(p, pb, c):
            return w_sb[:, ND * (NPB * p + pb) + c, :]

        wot_sb = [const.tile([128, SC], BF16, tag=f"wo{c}", name=f"wo{c}")
                  for c in range(ND)]
        for c in range(ND):
            nc.scalar.dma_start(out=wot_sb[c][:], in_=wo_t[c])

        bo_sb = const.tile([1, SC], F32, tag="bo1")
        nc.scalar.dma_start(out=bo_sb[:], in_=bo_col[:])
        bo_b = const.tile([128, SC], F32, tag="bob")
        nc.gpsimd.partition_broadcast(bo_b[:], bo_sb[:])

        ones_bf = const.tile([128, 1], BF16, tag="ones_bf")
        nc.vector.memset(ones_bf[:], 1.0)

        ident = const.tile([128, 128], BF16, tag="ident")
        nc.gpsimd.memset(ident[:], 1.0)
        nc.gpsimd.affine_select(out=ident[:], in_=ident[:], compare_op=GE,
                                fill=0.0, base=0, pattern=[[-1, 128]],
                                channel_multiplier=1)
        nc.gpsimd.affine_select(out=ident[:], in_=ident[:], compare_op=GE,
                                fill=0.0, base=0, pattern=[[1, 128]],
                                channel_multiplier=-1)

        ag_in = [dram.tile([128, S], BF16, tag=f"ag_in{pb}", name=f"ag_in{pb}")
                 for pb in range(NPB)]
        ag_out = [dram.tile([2, 128, S], BF16, tag=f"ag_out{pb}",
                            name=f"ag_out{pb}") for pb in range(NPB)]

        qt = [qtp.tile([128, S], BF16, tag=f"qt{pb}", name=f"qt{pb}")
              for pb in range(NPB)]
        kt = [ktp.tile([128, S], BF16, tag=f"kt{pb}", name=f"kt{pb}")
              for pb in range(NPB)]

        # ---- per-pblock: projections + attention + pair AllGather ----
        for pb in range(NPB):
            # projections: q, k into [128, S] bf16; v via transpose to
            # v01[h] [128 s_k, NK, 65] (64 V cols | ones col).
            vt = vtp.tile([128, S], BF16, tag="vt")
            for p in range(3):
                for j2 in range(2):
                    sl2 = slice(1024 * j2, 1024 * (j2 + 1))
                    ps = ps_mm.tile([128, 2 * SC], F32, tag="mm",
                                    name=f"pj{pb}_{p}_{j2}")
                    for j4 in range(2):
                        for c in range(ND):
                            nc.tensor.matmul(
                                ps[:, SC * j4:SC * (j4 + 1)],
                                lhsT=wq(p, pb, c),
                                rhs=et[c][:, 1024 * j2 + SC * j4:
                                          1024 * j2 + SC * (j4 + 1)],
                                start=(c == 0), stop=(c == ND - 1))
                    if p == 0:
                        nc.vector.tensor_copy(qt[pb][:, sl2], ps[:])
                    elif p == 1:
                        nc.vector.tensor_copy(kt[pb][:, sl2], ps[:])
                    else:
                        nc.scalar.copy(vt[:, sl2], ps[:])

            v01 = [vsb.tile([128, NK, 65], BF16, tag=f"v{h}", name=f"v{pb}_{h}")
                   for h in range(2)]
            for sk in range(NK):
                pt = ps_mm.tile([128, 128], BF16, tag="mm", name=f"tr{pb}_{sk}")
                nc.tensor.transpose(pt[:], vt[:, 128 * sk:128 * (sk + 1)],
                                    ident[:])
                for h in range(2):
                    nc.vector.tensor_copy(v01[h][:, sk, 0:64],
                                          pt[:, 64 * h:64 * (h + 1)])
                    nc.vector.tensor_copy(v01[h][:, sk, 64:65], ones_bf[:])

            # attention, one s_q chunk of 512 at a time
            for j in range(4):
                mtop = 4 * j + 4
                ctx_ps = [ps_ctx.tile([65, SC], F32, tag="ctx",
                                      name=f"ctx{pb}_{j}_{h}")
                          for h in range(2)]
                PIPE = 2
                exq = []

                def emit_scores(m, pb=pb, j=j, ctx_ps=ctx_ps, exq=exq):
                    # cols [0, c0) of each half are fully causal-masked
                    c0 = max(0, 128 * m - SC * j)
                    psc = ps_mm.tile([128, 2 * SC], F32, tag="mm",
                                     name=f"sc{pb}_{j}_{m}")
                    for h in range(2):
                        nc.tensor.matmul(
                            psc[:, SC * h + c0:SC * (h + 1)],
                            lhsT=kt[pb][64 * h:64 * (h + 1),
                                        128 * m:128 * (m + 1)],
                            rhs=qt[pb][64 * h:64 * (h + 1),
                                       SC * j + c0:SC * (j + 1)],
                            start=True, stop=True)
                    ex = exp_p.tile([128, 2 * SC], BF16, tag="ex",
                                    name=f"ex{pb}_{j}_{m}")
                    nc.scalar.activation(out=ex[:, c0:], in_=psc[:, c0:],
                                         func=EXP, scale=0.125)
                    if m >= 4 * j:  # diagonal tile: zero k>q entries
                        nc.gpsimd.affine_select(
                            out=ex[:], in_=ex[:], compare_op=GE, fill=0.0,
                            base=SC * j - 128 * m, pattern=[[0, 2], [1, SC]],
                            channel_multiplier=-1)
                    exq.append((m, ex))

                def emit_av(ctx_ps=ctx_ps, exq=exq, mtop=mtop):
                    m_av, ex = exq.pop(0)
                    for h in range(2):
                        nc.tensor.matmul(
                            ctx_ps[h][:], lhsT=v01[h][:, m_av, :],
                            rhs=ex[:, SC * h:SC * (h + 1)],
                            start=(m_av == 0), stop=(m_av == mtop - 1))

                for m in range(mtop):
                    emit_scores(m)
                    if len(exq) > PIPE:
                        emit_av()
                while exq:
                    emit_av()

                # free PSUM fast, then normalize off the critical path
                for h in range(2):
                    cu = cu_p.tile([64, SC], F32, tag="cu",
                                   name=f"cu{pb}_{j}_{h}")
                    nc.scalar.copy(cu[:], ctx_ps[h][0:64, :])
                    dn = rc_p.tile([1, SC], F32, tag="dn",
                                   name=f"dn{pb}_{j}_{h}")
                    nc.scalar.copy(dn[:], ctx_ps[h][64:65, :])
                    rc = rc_p.tile([1, SC], F32, tag="rc")
                    nc.vector.reciprocal_approx_fast(rc[:], dn[:])
                    rb = rb_p.tile([64, SC], F32, tag="rb")
                    nc.gpsimd.partition_broadcast(rb[:], rc[:])
                    cn = cn_p.tile([64, SC], BF16, tag="cn")
                    nc.vector.tensor_mul(cn[:], cu[:], rb[:])
                    nc.sync.dma_start(
                        out=ag_in[pb][64 * h:64 * (h + 1), SC * j:SC * (j + 1)],
                        in_=cn[:])

            nc.gpsimd.collective_compute(
                "AllGather", mybir.AluOpType.bypass,
                replica_groups=PAIRS,
                ins=[ag_in[pb].opt()], outs=[ag_out[pb].opt()])

        # ---- output projection: all 2048 rows x my 512 cols ----
        # concat chunk c = slot*4+pb: slot order == natural head order.
        cats = []
        for c in range(ND):
            slot, pb = c // NPB, c % NPB
            ct = cat_p.tile([128, S], BF16, tag="cat", name=f"cat{c}")
            eng = nc.sync if (c % 2 == 0) else nc.scalar
            eng.dma_start(out=ct[:], in_=ag_out[pb][slot])
            cats.append(ct)
        for sq in range(NK):
            po = ps_mm.tile([128, SC], F32, tag="mm", name=f"po{sq}")
            for c in range(ND):
                nc.tensor.matmul(
                    po[:], lhsT=cats[c][:, 128 * sq:128 * (sq + 1)],
                    rhs=wot_sb[c][:],
                    start=(c == 0), stop=(c == ND - 1))
            ob = ob_p.tile([128, SC], F32, tag="ob")
            nc.vector.tensor_add(ob[:], po[:], bo_b[:])
            nc.sync.dma_start(
                out=out_shard[128 * sq:128 * (sq + 1), :], in_=ob[:])


_NC_CACHE = None


def _get_nc():
    global _NC_CACHE
    if _NC_CACHE is None:
        _NC_CACHE = build()
    return _NC_CACHE


def kernel(embedded, Wq, Wk, Wv, Wo, bo, _trace=False):
    import ml_dtypes

    embedded = np.asarray(embedded, np.float32)
    W = np.stack([np.asarray(Wq), np.asarray(Wk), np.asarray(Wv)]
                 ).astype(np.float32)                       # [3, H, D, HD]
    Wo = np.asarray(Wo, np.float32)
    bo = np.asarray(bo, np.float32)

    # emb per batch: [D, S] chunked [ND, 128, S]
    emb_b = [np.ascontiguousarray(embedded[p].T).astype(ml_dtypes.bfloat16)
             .reshape(ND, 128, S) for p in range(B)]
    # w per role: heads hs..hs+8 -> [3, NPB, ND, 128, 128]
    w_r = []
    for r in range(2):
        w = W[:, 8 * r:8 * r + 8]                           # [3, 8, D, HD]
        w = w.reshape(3, NPB, 2, D, HD).transpose(0, 1, 3, 2, 4)
        w = np.ascontiguousarray(w).reshape(3, NPB, ND, 128, 128)
        w_r.append(w.astype(ml_dtypes.bfloat16))
    # wo per role: my 512 output cols -> [ND, 128, SC]
    wo_r = [np.ascontiguousarray(Wo[SC * r:SC * (r + 1), :].T)
            .astype(ml_dtypes.bfloat16).reshape(ND, 128, SC) for r in range(2)]
    bo_r = [bo[SC * r:SC * (r + 1)].reshape(1, SC) for r in range(2)]

    in_maps = []
    for c in range(NC_):
        p, r = c // 2, c % 2
        in_maps.append({
            "emb_t": emb_b[p],
            "w_qkv": w_r[r],
            "wo_t": wo_r[r],
            "bo_col": bo_r[r],
        })

    nc = _get_nc()
    res = run_bass_kernel_spmd(nc, in_maps, core_ids=list(range(NC_)),
                               trace=_trace)

    out = np.empty((B, S, D), np.float32)
    for c in range(NC_):
        p, r = c // 2, c % 2
        out[p, :, SC * r:SC * (r + 1)] = res.results[c]["out_shard"]
    if _trace:
        return out, res
    return out


# revision 8
# speedup vs baseline: 1.5142x; 1.1227x over previous
"""Multi-headed causal attention on 8 trn2 NeuronCores (Bass/Tile).

Sharding: batch per core-PAIR, heads split within the pair.
Core c: batch c//2, heads 8*(c%2) .. 8*(c%2)+8 (4 pblocks of 2 heads),
output columns 512*(c%2) .. +512 of its batch (col-split of Wo).

All per-core differences are INPUT DATA (emb of its batch, its 8 heads'
Wq/Wk/Wv, its 512 Wo columns + bias slice); the SPMD program is
identical, so cores never need role-dependent addressing.

Per core:
  - Q^T/K^T/V^T projections per pblock (2 heads stacked on 128
    partitions), contraction over D streamed from bf16 emb chunks.
  - scores transposed ([s_k, s_q]) in bf16, per-head K=64 matmuls at
    partition offset 64*h (PE tile_position) — no zero-padding.
  - exp on ScalarE (no max subtraction; logits ~N(0,1)); causal mask
    via one affine_select on GpSimd; always-masked leading columns of
    diagonal tiles skipped in matmul+exp.
  - AV with V padded to 65 cols (64 V | ones): softmax denominators
    fall out as row 64 of the context accumulator; normalize via
    fast-reciprocal + GpSimd partition broadcast.
  - Per-pblock 2-core AllGather (groups [[0,1],[2,3],[4,5],[6,7]])
    exchanges normalized context within the pair — only pair-local
    launch skew is absorbed, and each AG overlaps the next pblock's
    attention.
  - Output projection: my 512 Wo columns for ALL 2048 rows of the
    batch, contraction over the full concat dim read back from the
    AllGather outputs (slot order = natural head order, so no role
    dependence).
Proj and attention are interleaved per pblock so ScalarE exp work
overlaps the next pblock's projections; a 2-deep score->exp->AV
pipeline keeps the PE busy inside the attention loop.
"""
import sys

sys.path.insert(0, "/opt/trn_rl_repo")

import numpy as np

import concourse.bass as bass
import concourse.tile as tile
from concourse import bacc, mybir
from concourse.bass_utils import run_bass_kernel_spmd

B, S, D, H, HD = 4, 2048, 1024, 16, 64
NC_ = 8          # cores
NPB = 4          # pblocks per core (2 heads each -> 8 heads)
SC = 512         # s_q chunk (psum bank width in fp32)
NK = S // 128    # 16 s_k chunks of 128
ND = D // 128    # 8 contraction chunks of 128
F32 = mybir.dt.float32
BF16 = mybir.dt.bfloat16
EXP = mybir.ActivationFunctionType.Exp
GE = mybir.AluOpType.is_ge
PAIRS = [[0, 1], [2, 3], [4, 5], [6, 7]]


def build():
    nc = bacc.Bacc("TRN2", target_bir_lowering=False, debug=False, num_devices=NC_)

    emb_t = nc.dram_tensor("emb_t", [ND, 128, S], BF16, kind="ExternalInput").ap()
    w_qkv = nc.dram_tensor("w_qkv", [3, NPB, ND, 128, 128], BF16,
                           kind="ExternalInput").ap()
    wo_t = nc.dram_tensor("wo_t", [ND, 128, SC], BF16, kind="ExternalInput").ap()
    bo_col = nc.dram_tensor("bo_col", [1, SC], F32, kind="ExternalInput").ap()
    out_shard = nc.dram_tensor("out_shard", [S, SC], F32, kind="ExternalOutput").ap()

    with tile.TileContext(nc) as tc:
        _build_body(nc, tc, emb_t, w_qkv, wo_t, bo_col, out_shard)

    nc.compile()
    return nc


def _build_body(nc, tc, emb_t, w_qkv, wo_t, bo_col, out_shard):
    from contextlib import ExitStack

    ctx = ExitStack()
    with ctx:
        const = ctx.enter_context(tc.tile_pool(name="const", bufs=1))
        # "mm" slots are [128, 1024] fp32 (2 PSUM banks): 3x2 + ctx 2x1 = 8
        ps_mm = ctx.enter_context(tc.tile_pool(name="ps_mm", bufs=3, space="PSUM"))
        ps_ctx = ctx.enter_context(tc.tile_pool(name="ps_ctx", bufs=2, space="PSUM"))
        dram = ctx.enter_context(tc.tile_pool(name="dram", bufs=1, space="DRAM"))

        etp = ctx.enter_context(tc.tile_pool(name="etp", bufs=1))
        qtp = ctx.enter_context(tc.tile_pool(name="qtp", bufs=1))
        ktp = ctx.enter_context(tc.tile_pool(name="ktp", bufs=1))
        vtp = ctx.enter_context(tc.tile_pool(name="vtp", bufs=2))
        vsb = ctx.enter_context(tc.tile_pool(name="vsb", bufs=2))
        exp_p = ctx.enter_context(tc.tile_pool(name="exp_p", bufs=6))
        cu_p = ctx.enter_context(tc.tile_pool(name="cu_p", bufs=3))
        cn_p = ctx.enter_context(tc.tile_pool(name="cn_p", bufs=3))
        rc_p = ctx.enter_context(tc.tile_pool(name="rc_p", bufs=2))
        rb_p = ctx.enter_context(tc.tile_pool(name="rb_p", bufs=2))
        cat_p = ctx.enter_context(tc.tile_pool(name="cat_p", bufs=8))
        ob_p = ctx.enter_context(tc.tile_pool(name="ob_p", bufs=16))
        ob2_p = ctx.enter_context(tc.tile_pool(name="ob2_p", bufs=3))
        cats = {}

        # ---- input DMAs ----
        # emb per (j4, c) 512-col slices and weights per (p, pb) tiles,
        # interleaved so the first projection slot gates on ~1.3MB only.
        et = {}
        for c in range(ND):
            t = etp.tile([128, S], BF16, tag=f"et{c}", name=f"et{c}")
            et[c] = t
        w_t = {}
        for p in range(3):
            for pb in range(NPB):
                w_t[(p, pb)] = const.tile([128, ND, 128], BF16,
                                          tag=f"w{p}_{pb}", name=f"w{p}_{pb}")

        def emit_emb_j4(j4):
            sl = slice(SC * j4, SC * (j4 + 1))
            for c in range(ND):
                eng = nc.sync if (c % 2 == 0) else nc.scalar
                eng.dma_start(out=et[c][:, sl], in_=emb_t[c, :, sl])

        def emit_w_pb(pb):
            for p in range(3):
                i = NPB * p + pb
                eng = nc.sync if (p % 2 == 0) else nc.scalar
                eng.dma_start(
                    out=w_t[(p, pb)][:],
                    in_=bass.AP(tensor=w_qkv.tensor,
                                offset=(i * ND) * 128 * 128,
                                ap=[[128, 128], [16384, ND], [1, 128]]))

        emit_emb_j4(0)
        emit_emb_j4(1)
        emit_w_pb(0)
        emit_emb_j4(2)
        emit_emb_j4(3)
        for pb in range(1, NPB):
            emit_w_pb(pb)

        def wq(p, pb, c):
            return w_t[(p, pb)][:, c, :]

        wot_sb = [const.tile([128, SC], BF16, tag=f"wo{c}", name=f"wo{c}")
                  for c in range(ND)]
        for c in range(ND):
            nc.scalar.dma_start(out=wot_sb[c][:], in_=wo_t[c])

        bo_sb = const.tile([1, SC], F32, tag="bo1")
        nc.scalar.dma_start(out=bo_sb[:], in_=bo_col[:])
        bo_b = const.tile([128, SC], F32, tag="bob")
        nc.gpsimd.partition_broadcast(bo_b[:], bo_sb[:])

        ones_bf = const.tile([128, 1], BF16, tag="ones_bf")
        nc.vector.memset(ones_bf[:], 1.0)

        ident = const.tile([128, 128], BF16, tag="ident")
        nc.gpsimd.memset(ident[:], 1.0)
        nc.gpsimd.affine_select(out=ident[:], in_=ident[:], compare_op=GE,
                                fill=0.0, base=0, pattern=[[-1, 128]],
                                channel_multiplier=1)
        nc.gpsimd.affine_select(out=ident[:], in_=ident[:], compare_op=GE,
                                fill=0.0, base=0, pattern=[[1, 128]],
                                channel_multiplier=-1)

        ag_in = [dram.tile([128, S], BF16, tag=f"ag_in{pb}", name=f"ag_in{pb}")
                 for pb in range(NPB)]
        ag_out = [dram.tile([2, 128, S], BF16, tag=f"ag_out{pb}",
                            name=f"ag_out{pb}") for pb in range(NPB)]

        qt = [qtp.tile([128, S], BF16, tag=f"qt{pb}", name=f"qt{pb}")
              for pb in range(NPB)]
        kt = [ktp.tile([128, S], BF16, tag=f"kt{pb}", name=f"kt{pb}")
              for pb in range(NPB)]

        # ---- per-pblock: projections + attention + pair AllGather ----
        for pb in range(NPB):
            # projections: q, k into [128, S] bf16; v via transpose to
            # v01[h] [128 s_k, NK, 65] (64 V cols | ones col).
            vt = vtp.tile([128, S], BF16, tag="vt")
            for p in range(3):
                for j2 in range(2):
                    sl2 = slice(1024 * j2, 1024 * (j2 + 1))
                    ps = ps_mm.tile([128, 2 * SC], F32, tag="mm",
                                    name=f"pj{pb}_{p}_{j2}")
                    for j4 in range(2):
                        for c in range(ND):
                            nc.tensor.matmul(
                                ps[:, SC * j4:SC * (j4 + 1)],
                                lhsT=wq(p, pb, c),
                                rhs=et[c][:, 1024 * j2 + SC * j4:
                                          1024 * j2 + SC * (j4 + 1)],
                                start=(c == 0), stop=(c == ND - 1))
                    if p == 0:
                        nc.vector.tensor_copy(qt[pb][:, sl2], ps[:])
                    elif p == 1:
                        nc.vector.tensor_copy(kt[pb][:, sl2], ps[:])
                    else:
                        nc.vector.tensor_copy(vt[:, sl2], ps[:])

            v01 = [vsb.tile([128, NK, 65], BF16, tag=f"v{h}", name=f"v{pb}_{h}")
                   for h in range(2)]
            for sk in range(NK):
                pt = ps_mm.tile([128, 128], BF16, tag="mm", name=f"tr{pb}_{sk}")
                nc.tensor.transpose(pt[:], vt[:, 128 * sk:128 * (sk + 1)],
                                    ident[:])
                for h in range(2):
                    nc.vector.tensor_copy(v01[h][:, sk, 0:64],
                                          pt[:, 64 * h:64 * (h + 1)])
                    nc.vector.tensor_copy(v01[h][:, sk, 64:65], ones_bf[:])

            # attention, one s_q chunk of 512 at a time
            for j in range(4):
                mtop = 4 * j + 4
                ctx_ps = [ps_ctx.tile([65, SC], F32, tag="ctx",
                                      name=f"ctx{pb}_{j}_{h}")
                          for h in range(2)]
                PIPE = 2
                exq = []

                def emit_scores(m, pb=pb, j=j, ctx_ps=ctx_ps, exq=exq):
                    # cols [0, c0) of each half are fully causal-masked
                    c0 = max(0, 128 * m - SC * j)
                    psc = ps_mm.tile([128, 2 * SC], F32, tag="mm",
                                     name=f"sc{pb}_{j}_{m}")
                    for h in range(2):
                        nc.tensor.matmul(
                            psc[:, SC * h + c0:SC * (h + 1)],
                            lhsT=kt[pb][64 * h:64 * (h + 1),
                                        128 * m:128 * (m + 1)],
                            rhs=qt[pb][64 * h:64 * (h + 1),
                                       SC * j + c0:SC * (j + 1)],
                            start=True, stop=True)
                    ex = exp_p.tile([128, 2 * SC], BF16, tag="ex",
                                    name=f"ex{pb}_{j}_{m}")
                    nc.scalar.activation(out=ex[:, c0:], in_=psc[:, c0:],
                                         func=EXP, scale=0.125)
                    if m >= 4 * j:  # diagonal tile: zero k>q entries
                        nc.gpsimd.affine_select(
                            out=ex[:], in_=ex[:], compare_op=GE, fill=0.0,
                            base=SC * j - 128 * m, pattern=[[0, 2], [1, SC]],
                            channel_multiplier=-1)
                    exq.append((m, ex))

                def emit_av(ctx_ps=ctx_ps, exq=exq, mtop=mtop):
                    m_av, ex = exq.pop(0)
                    for h in range(2):
                        nc.tensor.matmul(
                            ctx_ps[h][:], lhsT=v01[h][:, m_av, :],
                            rhs=ex[:, SC * h:SC * (h + 1)],
                            start=(m_av == 0), stop=(m_av == mtop - 1))

                for m in range(mtop):
                    emit_scores(m)
                    if len(exq) > PIPE:
                        emit_av()
                while exq:
                    emit_av()

                # free PSUM fast, then normalize off the critical path
                for h in range(2):
                    cu = cu_p.tile([64, SC], F32, tag="cu",
                                   name=f"cu{pb}_{j}_{h}")
                    nc.vector.tensor_copy(cu[:], ctx_ps[h][0:64, :])
                    dn = rc_p.tile([1, SC], F32, tag="dn",
                                   name=f"dn{pb}_{j}_{h}")
                    nc.vector.tensor_copy(dn[:], ctx_ps[h][64:65, :])
                    rc = rc_p.tile([1, SC], F32, tag="rc")
                    nc.vector.reciprocal_approx_fast(rc[:], dn[:])
                    rb = rb_p.tile([64, SC], F32, tag="rb")
                    nc.gpsimd.partition_broadcast(rb[:], rc[:])
                    cn = cn_p.tile([64, SC], BF16, tag="cn")
                    nc.vector.tensor_mul(cn[:], cu[:], rb[:])
                    nc.sync.dma_start(
                        out=ag_in[pb][64 * h:64 * (h + 1), SC * j:SC * (j + 1)],
                        in_=cn[:])

            nc.gpsimd.collective_compute(
                "AllGather", mybir.AluOpType.bypass,
                replica_groups=PAIRS,
                ins=[ag_in[pb].opt()], outs=[ag_out[pb].opt()])
            # stage this pblock's concat chunks (both slots) while later
            # pblocks' attention runs
            for slot in range(2):
                c = slot * NPB + pb
                ct = cat_p.tile([128, S], BF16, tag="cat", name=f"cat{c}")
                eng = nc.sync if (slot == 0) else nc.scalar
                eng.dma_start(out=ct[:], in_=ag_out[pb][slot])
                cats[c] = ct

        # ---- output projection: all 2048 rows x my 512 cols ----
        # concat chunk c = slot*4+pb: slot order == natural head order.
        # pass 1 (chunks of pblocks 0-2, available after AG(2)) overlaps
        # AG(3)'s flight; pass 2 adds pblock-3 chunks.
        P1 = [c for c in range(ND) if c % NPB != NPB - 1]
        P2 = [c for c in range(ND) if c % NPB == NPB - 1]
        obs = {}
        for sq in range(NK):
            po = ps_mm.tile([128, SC], F32, tag="mm", name=f"po{sq}")
            for i, c in enumerate(P1):
                nc.tensor.matmul(
                    po[:], lhsT=cats[c][:, 128 * sq:128 * (sq + 1)],
                    rhs=wot_sb[c][:],
                    start=(i == 0), stop=(i == len(P1) - 1))
            ob = ob_p.tile([128, SC], BF16, tag="ob", name=f"ob{sq}")
            nc.vector.tensor_add(ob[:], po[:], bo_b[:])
            obs[sq] = ob
        for sq in range(NK):
            po = ps_mm.tile([128, SC], F32, tag="mm", name=f"po2_{sq}")
            for i, c in enumerate(P2):
                nc.tensor.matmul(
                    po[:], lhsT=cats[c][:, 128 * sq:128 * (sq + 1)],
                    rhs=wot_sb[c][:],
                    start=(i == 0), stop=(i == len(P2) - 1))
            ob2 = ob2_p.tile([128, SC], F32, tag="ob2", name=f"ob2_{sq}")
            nc.vector.tensor_add(ob2[:], po[:], obs[sq][:])
            nc.sync.dma_start(
                out=out_shard[128 * sq:128 * (sq + 1), :], in_=ob2[:])


_NC_CACHE = None


def _get_nc():
    global _NC_CACHE
    if _NC_CACHE is None:
        _NC_CACHE = build()
    return _NC_CACHE


def kernel(embedded, Wq, Wk, Wv, Wo, bo, _trace=False):
    import ml_dtypes

    embedded = np.asarray(embedded, np.float32)
    W = np.stack([np.asarray(Wq), np.asarray(Wk), np.asarray(Wv)]
                 ).astype(np.float32)                       # [3, H, D, HD]
    Wo = np.asarray(Wo, np.float32)
    bo = np.asarray(bo, np.float32)

    # emb per batch: [D, S] chunked [ND, 128, S]
    emb_b = [np.ascontiguousarray(embedded[p].T).astype(ml_dtypes.bfloat16)
             .reshape(ND, 128, S) for p in range(B)]
    # w per role: heads hs..hs+8 -> [3, NPB, ND, 128, 128]
    w_r = []
    for r in range(2):
        w = W[:, 8 * r:8 * r + 8]                           # [3, 8, D, HD]
        w = w.reshape(3, NPB, 2, D, HD).transpose(0, 1, 3, 2, 4)
        w = np.ascontiguousarray(w).reshape(3, NPB, ND, 128, 128)
        w_r.append(w.astype(ml_dtypes.bfloat16))
    # wo per role: my 512 output cols -> [ND, 128, SC]
    wo_r = [np.ascontiguousarray(Wo[SC * r:SC * (r + 1), :].T)
            .astype(ml_dtypes.bfloat16).reshape(ND, 128, SC) for r in range(2)]
    bo_r = [bo[SC * r:SC * (r + 1)].reshape(1, SC) for r in range(2)]

    in_maps = []
    for c in range(NC_):
        p, r = c // 2, c % 2
        in_maps.append({
            "emb_t": emb_b[p],
            "w_qkv": w_r[r],
            "wo_t": wo_r[r],
            "bo_col": bo_r[r],
        })

    nc = _get_nc()
    res = run_bass_kernel_spmd(nc, in_maps, core_ids=list(range(NC_)),
                               trace=_trace)

    out = np.empty((B, S, D), np.float32)
    for c in range(NC_):
        p, r = c // 2, c % 2
        out[p, :, SC * r:SC * (r + 1)] = res.results[c]["out_shard"]
    if _trace:
        return out, res
    return out
